# revision 1
# baseline (speedup 1.0000x reference)
"""Causal multi-head attention (B=2, S=2048, D=1024, H=16, hd=64) on 8 trn2 cores.

Sharding: core c handles batch b = c//4 and head group hg = c%4 (4 heads each).
Each core computes its Q/K/V shard (tensor-parallel columns of W_qkv), causal
attention for its 4 heads with scores held transposed ([s_k, s_q] so the PV
matmul needs no on-chip transposes), and a partial output projection over its
256 rows of W_proj. The host sums the 4 partials per batch and adds the exact
bias terms (softmax rows sum to 1, so attn@(V + 1 bv^T) = attn@V + bv^T; the
b_qkv V-slice and b_proj are applied on the host).

Matmul operands are bf16 (PSUM accumulation fp32); x is transposed/cast on the
host as part of sharding so the device needs no transposes at all.
"""

import numpy as np
import ml_dtypes
from contextlib import ExitStack

B, S, D, H = 2, 2048, 1024, 16
HD = 64
NCORES = 8
FPC = 256  # features per core (4 heads x 64)

_CACHE = {}


def _build():
    import concourse.bacc as bacc
    import concourse.tile as tile
    import concourse.mybir as mybir

    f32 = mybir.dt.float32
    bf16 = mybir.dt.bfloat16
    AF = mybir.ActivationFunctionType

    nc = bacc.Bacc("TRN2", target_bir_lowering=False, debug=False, num_devices=NCORES)

    xT = nc.dram_tensor("xT", [D, S], bf16, kind="ExternalInput").ap()
    wq = nc.dram_tensor("wq", [D, FPC], bf16, kind="ExternalInput").ap()
    wk = nc.dram_tensor("wk", [D, FPC], bf16, kind="ExternalInput").ap()
    wv = nc.dram_tensor("wv", [D, FPC], bf16, kind="ExternalInput").ap()
    wp = nc.dram_tensor("wp", [FPC, D], bf16, kind="ExternalInput").ap()
    bqk = nc.dram_tensor("bqk", [128, 4], f32, kind="ExternalInput").ap()
    maskT = nc.dram_tensor("maskT", [128, 128], bf16, kind="ExternalInput").ap()
    out = nc.dram_tensor("out", [S, D], f32, kind="ExternalOutput").ap()

    with tile.TileContext(nc) as tc:
        with ExitStack() as ctx:
            _body(ctx, tc, mybir, out, xT, wq, wk, wv, wp, bqk, maskT)

    nc.compile()
    return nc


def _body(ctx, tc, mybir, out, xT, wq, wk, wv, wp, bqk, maskT):
    nc = tc.nc
    f32 = mybir.dt.float32
    bf16 = mybir.dt.bfloat16
    AF = mybir.ActivationFunctionType
    NK = D // 128   # 8 contraction tiles for qkv/proj-input dim
    NS = S // 128   # 16 sequence tiles

    sb = ctx.enter_context(tc.tile_pool(name="sb", bufs=1))

    xt_a = sb.tile([128, NK * S], bf16, name="xta", tag="xta")
    xt_t = [xt_a[:, k * S:(k + 1) * S] for k in range(NK)]
    wq_a = sb.tile([128, NK * FPC], bf16, name="wqa", tag="wqa")
    wq_t = [wq_a[:, k * FPC:(k + 1) * FPC] for k in range(NK)]
    wk_a = sb.tile([128, NK * FPC], bf16, name="wka", tag="wka")
    wk_t = [wk_a[:, k * FPC:(k + 1) * FPC] for k in range(NK)]
    wv_a = sb.tile([128, NK * FPC], bf16, name="wva", tag="wva")
    wv_t = [wv_a[:, k * FPC:(k + 1) * FPC] for k in range(NK)]
    wp_a = sb.tile([128, 2 * D], bf16, name="wpa", tag="wpa")
    wp_t = [wp_a[:, k * D:(k + 1) * D] for k in range(2)]
    qt_t = [sb.tile([128, S], bf16, name=f"qtt{f}", tag=f"qtt{f}") for f in range(2)]
    kt_t = [sb.tile([128, S], bf16, name=f"ktt{f}", tag=f"ktt{f}") for f in range(2)]
    v_t = [sb.tile([128, 4 * 65], bf16, name=f"vt{s}", tag=f"vt{s}") for s in range(NS)]
    ot_t = [sb.tile([128, S], bf16, name=f"ott{f}", tag=f"ott{f}") for f in range(2)]
    bqk_t = sb.tile([128, 4], f32, name="bqkt", tag="bqkt")
    mask_t = sb.tile([128, 128], bf16, name="maskt", tag="maskt")

    p_pool = ctx.enter_context(tc.tile_pool(name="pp", bufs=4))
    rc_pool = ctx.enter_context(tc.tile_pool(name="rcp", bufs=2))
    oo_pool = ctx.enter_context(tc.tile_pool(name="oop", bufs=3))

    # ---- input DMAs, few big transfers, critical-path first: the first
    # attention pair needs only bqk/mask, wq/wk and the s<1024 half of x^T
    nc.sync.dma_start(bqk_t[:], bqk[:])
    nc.sync.dma_start(mask_t[:], maskT[:])
    nc.sync.dma_start(wq_a.rearrange("p (k f) -> p k f", k=NK),
                      wq.rearrange("(k p) f -> p k f", p=128))
    nc.sync.dma_start(wk_a.rearrange("p (k f) -> p k f", k=NK),
                      wk.rearrange("(k p) f -> p k f", p=128))
    xt3 = xt_a.rearrange("p (k s) -> p k s", k=NK)
    xs3 = xT.rearrange("(k p) s -> p k s", p=128)
    nc.sync.dma_start(xt3[:, :, 0:1024], xs3[:, :, 0:1024])
    nc.sync.dma_start(wv_a.rearrange("p (k f) -> p k f", k=NK),
                      wv.rearrange("(k p) f -> p k f", p=128))
    nc.sync.dma_start(xt3[:, :, 1024:2048], xs3[:, :, 1024:2048])
    nc.sync.dma_start(wp_a.rearrange("p (k f) -> p k f", k=2),
                      wp.rearrange("(k p) f -> p k f", p=128))

    # Unified PSUM pools for every phase (no phase barriers): "sc" slots are
    # 2 banks each x2, "pv" slots 2 banks each x2 -> 8 banks total.
    scp = ctx.enter_context(tc.tile_pool(name="ps_sc", bufs=2, space="PSUM"))
    pvp = ctx.enter_context(tc.tile_pool(name="ps_pv", bufs=2, space="PSUM"))

    def qkt_group(dst, w_t, bcol, f, c2):
        """One [128,1024] accumulation group of the Q^T/K^T projection."""
        ps = scp.tile([128, 1024], f32, name="sc", tag="sc", bufs=2)
        for k in range(NK):
            for sp in range(2):
                nc.tensor.matmul(
                    ps[:, sp * 512:(sp + 1) * 512],
                    w_t[k][:, f * 128:(f + 1) * 128],
                    xt_t[k][:, c2 * 1024 + sp * 512: c2 * 1024 + (sp + 1) * 512],
                    start=(k == 0), stop=(k == NK - 1),
                )
        nc.vector.tensor_scalar_add(
            dst[f][:, c2 * 1024:(c2 + 1) * 1024], ps[:],
            bqk_t[:, bcol + f: bcol + f + 1],
        )

    def v_group(s):
        psv = scp.tile([128, FPC], f32, name="sc", tag="sc", bufs=2)
        for k in range(NK):
            nc.tensor.matmul(
                psv[:],
                xt_t[k][:, s * 128:(s + 1) * 128],
                wv_t[k][:],
                start=(k == 0), stop=(k == NK - 1),
            )
        v3 = v_t[s].rearrange("p (h c) -> p h c", h=4)
        nc.vector.tensor_copy(v3[:, :, 0:64],
                              psv.rearrange("p (h c) -> p h c", h=4)[:])
        nc.vector.memset(v3[:, :, 64:65], 1.0)

    class AttnUnit:
        """Causal attention for head h over queries [half*1024, +1024)."""

        def __init__(self, h, half):
            self.h, self.half = h, half
            self.hp, self.hh = h // 2, h % 2
            self.r0 = self.hh * 64
            self.q0 = half * 1024
            self.ki_n = NS // 2 * (half + 1)
            self.pv = pvp.tile([128, 1024], f32, name="pv", tag="pv", bufs=2)

        def emit_scores(self, ki):
            q0, r0 = self.q0, self.r0
            qt, kt = qt_t[self.hp], kt_t[self.hp]
            qs = max(ki * 128, q0)   # first unmasked q for this k block
            a0 = qs - q0             # local col offset in the 1024 tile
            self.a0 = a0
            self.diag = ki * 128 >= q0   # diagonal block lives in this half
            self.spans = [(a0, 512), (512, 1024)] if a0 < 512 else [(a0, 1024)]
            self.sc = scp.tile([128, 1024], f32, name="sc", tag="sc", bufs=2)
            for (a, b) in self.spans:
                nc.tensor.matmul(
                    self.sc[:, a:b],
                    kt[r0:r0 + 64, ki * 128:(ki + 1) * 128],
                    qt[r0:r0 + 64, q0 + a:q0 + b],
                    start=True, stop=True,
                )

        def emit_exp(self, ki):
            a0 = self.a0
            self.P = p_pool.tile([128, 1024], bf16, name="P", tag="P", bufs=6)
            nc.scalar.activation(self.P[:, a0:1024], self.sc[:, a0:1024], AF.Exp,
                                 scale=float(HD) ** -0.5)
            if self.diag:  # causal mask on the diagonal block
                nc.vector.tensor_mul(self.P[:, a0:a0 + 128],
                                     self.P[:, a0:a0 + 128], mask_t[:])

        def emit_pv(self, ki):
            for (a, b) in self.spans:
                # last k-block contributing to this psum bank
                last_ki = min(self.ki_n - 1, (self.q0 + b - 1) // 128)
                nc.tensor.matmul(
                    self.pv[0:65, a:b],
                    v_t[ki][:, self.h * 65:self.h * 65 + 65],
                    self.P[:, a:b],
                    start=(ki == 0), stop=(ki == last_ki),
                )

        def finish(self):
            pv = self.pv
            dcp = rc_pool.tile([1, 1024], f32, name="dcp", tag="dcp", bufs=2)
            nc.vector.tensor_copy(dcp[:], pv[64:65, 0:1024])
            rcp = rc_pool.tile([1, 1024], f32, name="rcp", tag="rcp", bufs=2)
            nc.vector.reciprocal_approx_fast(rcp[:], dcp[:])
            rbc = rc_pool.tile([64, 1024], f32, name="rbc", tag="rbc", bufs=2)
            nc.gpsimd.partition_broadcast(rbc[:], rcp[:], channels=64)
            nc.vector.tensor_mul(
                ot_t[self.hp][self.r0:self.r0 + 64, self.q0:self.q0 + 1024],
                pv[0:64, :], rbc[:],
            )

    def attn_pair(ha, hb, half, fillers=()):
        """Two heads' units interleaved at ki granularity (two chains in
        flight hide the scores->exp->PV latency). fillers[ki] is a list of
        thunks emitting independent PE work woven between iterations."""
        ua, ub = AttnUnit(ha, half), AttnUnit(hb, half)
        for ki in range(ua.ki_n):
            # adjacent scores MMs land in different PE row groups (heads at
            # partition 0 and 64) and execute concurrently in the array
            ua.emit_scores(ki)
            ub.emit_scores(ki)
            ua.emit_exp(ki)
            ub.emit_exp(ki)
            ua.emit_pv(ki)
            ub.emit_pv(ki)
            if ki < len(fillers):
                for fn in fillers[ki]:
                    fn()
        ua.finish()
        ub.finish()

    oo_box = {}

    def proj_group(s):
        pj = scp.tile([128, 1024], f32, name="sc", tag="sc", bufs=2)
        for nh in range(2):
            for k2 in range(2):
                nc.tensor.matmul(
                    pj[:, nh * 512:(nh + 1) * 512],
                    ot_t[k2][:, s * 128:(s + 1) * 128],
                    wp_t[k2][:, nh * 512:(nh + 1) * 512],
                    start=(k2 == 0), stop=(k2 == 1),
                )
        if s % 2 == 0:
            oo_box[0] = oo_pool.tile([128, 2 * D], f32, name="oo", tag="oo",
                                     bufs=2)
        oo = oo_box[0]
        nc.vector.tensor_copy(oo[:, (s % 2) * D:(s % 2 + 1) * D], pj[:])
        if s % 2 == 1:
            nc.sync.dma_start(
                out[(s - 1) * 128:(s + 1) * 128, :].rearrange(
                    "(g p) n -> p g n", p=128),
                oo.rearrange("p (g n) -> p g n", g=2))

    # Program order = scheduler priority. Prelude computes the f0 tiles of
    # Q^T/K^T plus all of V (PE-dense, warms HAM); the f1 tiles are emitted
    # as PE filler between the first attention stages (which are ACT-paced);
    # proj of a finished q-half fills the last stage's gaps.
    from functools import partial

    qkt_group(qt_t, wq_t, 0, 0, 0)
    qkt_group(kt_t, wk_t, 2, 0, 0)
    qkt_group(qt_t, wq_t, 0, 0, 1)
    qkt_group(kt_t, wk_t, 2, 0, 1)
    v_group(0)
    v_group(1)

    # weave V[2..7] (needed by this pair one ki ahead) and the f1 c2=0
    # Q^T/K^T groups (needed by the NEXT pair) into the first pair
    attn_pair(0, 1, 0, fillers=[
        [partial(v_group, 2), partial(v_group, 3)],
        [partial(v_group, 4), partial(v_group, 5)],
        [partial(v_group, 6), partial(v_group, 7)],
        [partial(qkt_group, qt_t, wq_t, 0, 1, 0)],
        [partial(qkt_group, kt_t, wk_t, 2, 1, 0)],
    ])
    # V[8..15] (needed by the half-1 pairs) woven into the second pair
    attn_pair(2, 3, 0, fillers=[
        [partial(v_group, 8 + ki)] for ki in range(8)])
    # f1 c2=1 (needed by pair(2,3,1)) + proj of the finished half 0
    attn_pair(0, 1, 1, fillers=[
        [partial(qkt_group, qt_t, wq_t, 0, 1, 1)],
        [partial(qkt_group, kt_t, wk_t, 2, 1, 1)],
        [partial(proj_group, 0)],
        [partial(proj_group, 1)],
        [partial(proj_group, 2)],
    ])
    attn_pair(2, 3, 1, fillers=[
        [partial(proj_group, s + 3)] if s < 5 else [] for s in range(16)])
    for s in range(8, NS):
        proj_group(s)


def _in_maps(x, W_qkv, b_qkv, W_proj):
    bf = ml_dtypes.bfloat16
    maps = []
    # multiplicative causal mask for the transposed diag block: keep k<=q
    mask = np.triu(np.ones((128, 128), np.float32)).astype(bf)
    for core in range(NCORES):
        b, hg = core // 4, core % 4
        cs = slice(hg * FPC, (hg + 1) * FPC)
        bq = b_qkv[cs].astype(np.float32)
        bk = b_qkv[D + hg * FPC: D + (hg + 1) * FPC].astype(np.float32)
        maps.append({
            "xT": np.ascontiguousarray(x[b].T).astype(bf),
            "wq": np.ascontiguousarray(W_qkv[:, cs]).astype(bf),
            "wk": np.ascontiguousarray(W_qkv[:, D + hg * FPC: D + (hg + 1) * FPC]).astype(bf),
            "wv": np.ascontiguousarray(W_qkv[:, 2 * D + hg * FPC: 2 * D + (hg + 1) * FPC]).astype(bf),
            "wp": np.ascontiguousarray(W_proj[hg * FPC:(hg + 1) * FPC, :]).astype(bf),
            "bqk": np.ascontiguousarray(
                np.stack([bq[0:128], bq[128:256], bk[0:128], bk[128:256]], axis=1)),
            "maskT": mask,
        })
    return maps


def get_nc():
    if "nc" not in _CACHE:
        _CACHE["nc"] = _build()
    return _CACHE["nc"]


def _postprocess(partials, b_qkv, W_proj, b_proj):
    out = np.zeros((B, S, D), np.float32)
    for core in range(NCORES):
        out[core // 4] += partials[core]
    bv = np.asarray(b_qkv, np.float32)[2 * D:3 * D]
    out += bv @ np.asarray(W_proj, np.float32) + np.asarray(b_proj, np.float32)
    return out


def kernel(x, W_qkv, b_qkv, W_proj, b_proj, _trace=False):
    from concourse.bass_utils import run_bass_kernel_spmd

    x = np.asarray(x, np.float32)
    W_qkv = np.asarray(W_qkv, np.float32)
    b_qkv = np.asarray(b_qkv, np.float32)
    W_proj = np.asarray(W_proj, np.float32)
    b_proj = np.asarray(b_proj, np.float32)

    nc = get_nc()
    maps = _in_maps(x, W_qkv, b_qkv, W_proj)
    res = run_bass_kernel_spmd(nc, maps, list(range(NCORES)), trace=_trace)
    _CACHE["last_result"] = res
    partials = [res.results[c]["out"] for c in range(NCORES)]
    return _postprocess(partials, b_qkv, W_proj, b_proj)



# revision 6
# speedup vs baseline: 1.0120x; 1.0120x over previous
"""Causal multi-head attention (B=2, S=2048, D=1024, H=16, hd=64) on 8 trn2 cores.

Sharding: core c handles batch b = c//4 and head group hg = c%4 (4 heads each).
Each core computes its Q/K/V shard (tensor-parallel columns of W_qkv), causal
attention for its 4 heads with scores held transposed ([s_k, s_q] so the PV
matmul needs no on-chip transposes), and a partial output projection over its
256 rows of W_proj. The host sums the 4 partials per batch and adds the exact
bias terms (softmax rows sum to 1, so attn@(V + 1 bv^T) = attn@V + bv^T; the
b_qkv V-slice and b_proj are applied on the host).

Schedule: input DMAs are split across four engine issue queues so the first
projection matmul starts ~4us in. Attention is software-pipelined: the PV
matmuls for step ki are emitted one iteration after the scores for ki, so the
exp (ACT) -> mask (Pool) latency never stalls the PE. The softmax divide is
done per 512-column PSUM bank as soon as that bank's accumulation stops,
which lets the output projection of the last q-ranges weave into the final
attention chain instead of trailing it. PSUM drains stay on DVE (gpsimd has
no PSUM port); SBUF-only work (causal mask, denominator broadcast, V spread)
runs on the otherwise-idle gpsimd.
"""

import numpy as np
import ml_dtypes
from contextlib import ExitStack

B, S, D, H = 2, 2048, 1024, 16
HD = 64
NCORES = 8
FPC = 256  # features per core (4 heads x 64)

_CACHE = {}


def _build():
    import concourse.bacc as bacc
    import concourse.tile as tile
    import concourse.mybir as mybir

    f32 = mybir.dt.float32
    bf16 = mybir.dt.bfloat16

    nc = bacc.Bacc("TRN2", target_bir_lowering=False, debug=False, num_devices=NCORES)

    xT = nc.dram_tensor("xT", [D, S], bf16, kind="ExternalInput").ap()
    wq = nc.dram_tensor("wq", [D, FPC], bf16, kind="ExternalInput").ap()
    wk = nc.dram_tensor("wk", [D, FPC], bf16, kind="ExternalInput").ap()
    wv = nc.dram_tensor("wv", [D, FPC], bf16, kind="ExternalInput").ap()
    wp = nc.dram_tensor("wp", [FPC, D], bf16, kind="ExternalInput").ap()
    bqk = nc.dram_tensor("bqk", [128, 4], f32, kind="ExternalInput").ap()
    maskT = nc.dram_tensor("maskT", [128, 128], bf16, kind="ExternalInput").ap()
    out = nc.dram_tensor("out", [S, D], bf16, kind="ExternalOutput").ap()

    with tile.TileContext(nc) as tc:
        with ExitStack() as ctx:
            _body(ctx, tc, mybir, out, xT, wq, wk, wv, wp, bqk, maskT)

    nc.compile()
    return nc


def _body(ctx, tc, mybir, out, xT, wq, wk, wv, wp, bqk, maskT):
    nc = tc.nc
    f32 = mybir.dt.float32
    bf16 = mybir.dt.bfloat16
    AF = mybir.ActivationFunctionType
    NK = D // 128   # 8 contraction tiles for qkv/proj-input dim
    NS = S // 128   # 16 sequence tiles

    sb = ctx.enter_context(tc.tile_pool(name="sb", bufs=1))

    xt_a = sb.tile([128, NK * S], bf16, name="xta", tag="xta")
    xt_t = [xt_a[:, k * S:(k + 1) * S] for k in range(NK)]
    wq_a = sb.tile([128, NK * FPC], bf16, name="wqa", tag="wqa")
    wq_t = [wq_a[:, k * FPC:(k + 1) * FPC] for k in range(NK)]
    wk_a = sb.tile([128, NK * FPC], bf16, name="wka", tag="wka")
    wk_t = [wk_a[:, k * FPC:(k + 1) * FPC] for k in range(NK)]
    wv_a = sb.tile([128, NK * FPC], bf16, name="wva", tag="wva")
    wv_t = [wv_a[:, k * FPC:(k + 1) * FPC] for k in range(NK)]
    wp_a = sb.tile([128, 2 * D], bf16, name="wpa", tag="wpa")
    wp_t = [wp_a[:, k * D:(k + 1) * D] for k in range(2)]
    qt_t = [sb.tile([128, S], bf16, name=f"qtt{f}", tag=f"qtt{f}") for f in range(2)]
    kt_t = [sb.tile([128, S], bf16, name=f"ktt{f}", tag=f"ktt{f}") for f in range(2)]
    v_t = [sb.tile([128, 4 * 65], bf16, name=f"vt{s}", tag=f"vt{s}") for s in range(NS)]
    ot_t = [sb.tile([128, S], bf16, name=f"ott{f}", tag=f"ott{f}") for f in range(2)]
    bqk_t = sb.tile([128, 4], f32, name="bqkt", tag="bqkt")
    mask_t = sb.tile([128, 128], bf16, name="maskt", tag="maskt")

    p_pool = ctx.enter_context(tc.tile_pool(name="pp", bufs=1))
    vs_pool = ctx.enter_context(tc.tile_pool(name="vsp", bufs=2))
    rc_pool = ctx.enter_context(tc.tile_pool(name="rcp", bufs=4))
    oo_pool = ctx.enter_context(tc.tile_pool(name="oop", bufs=2))

    # ---- input DMAs split across four issue queues; each queue's order puts
    # the first attention pair's dependencies first.
    xt3 = xt_a.rearrange("p (k s) -> p k s", k=NK)
    xs3 = xT.rearrange("(k p) s -> p k s", p=128)
    # DMA issue is only possible from SP (sync), Activation (scalar), gpsimd.
    # sync: tiny consts, then x first-half k0-3, then x second-half k0-3
    nc.sync.dma_start(bqk_t[:], bqk[:])
    nc.sync.dma_start(mask_t[:], maskT[:])
    nc.sync.dma_start(xt3[:, 0:4, 0:1024], xs3[:, 0:4, 0:1024])
    nc.sync.dma_start(xt3[:, 0:4, 1024:2048], xs3[:, 0:4, 1024:2048])
    # scalar: wq, wk, then wv
    nc.scalar.dma_start(wq_a.rearrange("p (k f) -> p k f", k=NK),
                        wq.rearrange("(k p) f -> p k f", p=128))
    nc.scalar.dma_start(wk_a.rearrange("p (k f) -> p k f", k=NK),
                        wk.rearrange("(k p) f -> p k f", p=128))
    nc.scalar.dma_start(wv_a.rearrange("p (k f) -> p k f", k=NK),
                        wv.rearrange("(k p) f -> p k f", p=128))
    # gpsimd: x first-half k4-7, x second-half k4-7, then wp
    nc.gpsimd.dma_start(xt3[:, 4:8, 0:1024], xs3[:, 4:8, 0:1024])
    nc.gpsimd.dma_start(xt3[:, 4:8, 1024:2048], xs3[:, 4:8, 1024:2048])
    nc.gpsimd.dma_start(wp_a.rearrange("p (k f) -> p k f", k=2),
                        wp.rearrange("(k p) f -> p k f", p=128))

    # PSUM: "sc" ring (scores + all filler groups) 2 x [128,1024] = 4 banks,
    # "pv" accumulators 2 x [128,1024] = 4 banks.
    scp = ctx.enter_context(tc.tile_pool(name="ps_sc", bufs=2, space="PSUM"))
    pvp = ctx.enter_context(tc.tile_pool(name="ps_pv", bufs=2, space="PSUM"))

    def qkt_group(dst, w_t, bcol, f, c2):
        """One [128,1024] accumulation group of the Q^T/K^T projection."""
        ps = scp.tile([128, 1024], f32, name="sc", tag="sc", bufs=2)
        for k in range(NK):
            for sp in range(2):
                nc.tensor.matmul(
                    ps[:, sp * 512:(sp + 1) * 512],
                    w_t[k][:, f * 128:(f + 1) * 128],
                    xt_t[k][:, c2 * 1024 + sp * 512: c2 * 1024 + (sp + 1) * 512],
                    start=(k == 0), stop=(k == NK - 1),
                )
        nc.vector.tensor_scalar_add(
            dst[f][:, c2 * 1024:(c2 + 1) * 1024], ps[:],
            bqk_t[:, bcol + f: bcol + f + 1],
        )

    def v_group(s):
        psv = scp.tile([128, FPC], f32, name="sc", tag="sc", bufs=2)
        for k in range(NK):
            nc.tensor.matmul(
                psv[:],
                xt_t[k][:, s * 128:(s + 1) * 128],
                wv_t[k][:],
                start=(k == 0), stop=(k == NK - 1),
            )
        v3 = v_t[s].rearrange("p (h c) -> p h c", h=4)
        nc.vector.tensor_copy(v3[:, :, 0:64],
                              psv.rearrange("p (h c) -> p h c", h=4)[:])
        nc.vector.memset(v3[:, :, 64:65], 1.0)

    class AttnUnit:
        """Causal attention for head h over queries [half*1024, +1024)."""

        def __init__(self, h, half):
            self.h, self.half = h, half
            self.hp, self.hh = h // 2, h % 2
            self.r0 = self.hh * 64
            self.q0 = half * 1024
            self.ki_n = NS // 2 * (half + 1)
            self.fin0_ki = (self.q0 + 511) // 128  # bank0 stops after this ki
            self.pv = pvp.tile([128, 1024], f32, name="pv", tag="pv", bufs=2)
            self.P = {}
            self.spans = {}

        def emit_scores(self, ki):
            q0, r0 = self.q0, self.r0
            qt, kt = qt_t[self.hp], kt_t[self.hp]
            qs = max(ki * 128, q0)   # first unmasked q for this k block
            a0 = qs - q0             # local col offset in the 1024 tile
            diag = ki * 128 >= q0    # diagonal block lives in this half
            spans = [(a0, 512), (512, 1024)] if a0 < 512 else [(a0, 1024)]
            self.spans[ki] = (a0, diag, spans)
            sc = scp.tile([128, 1024], f32, name="sc", tag="sc", bufs=2)
            for (a, b) in spans:
                nc.tensor.matmul(
                    sc[:, a:b],
                    kt[r0:r0 + 64, ki * 128:(ki + 1) * 128],
                    qt[r0:r0 + 64, q0 + a:q0 + b],
                    start=True, stop=True,
                )
            self.sc = sc

        def emit_exp(self, ki):
            a0, diag, _ = self.spans[ki]
            P = p_pool.tile([128, 1024], bf16, name="P", tag="P", bufs=6)
            nc.scalar.activation(P[:, a0:1024], self.sc[:, a0:1024], AF.Exp,
                                 scale=float(HD) ** -0.5)
            if diag:  # causal mask on the diagonal block
                nc.vector.tensor_mul(P[:, a0:a0 + 128],
                                     P[:, a0:a0 + 128], mask_t[:])
            self.P[ki] = P

        def emit_pv(self, ki):
            _, _, spans = self.spans[ki]
            P = self.P.pop(ki)
            for (a, b) in spans:
                # last k-block contributing to this psum bank
                last_ki = min(self.ki_n - 1, (self.q0 + b - 1) // 128)
                nc.tensor.matmul(
                    self.pv[0:65, a:b],
                    v_t[ki][:, self.h * 65:self.h * 65 + 65],
                    P[:, a:b],
                    start=(ki == 0), stop=(ki == last_ki),
                )

        def finish_span(self, a, b):
            """Divide pv rows by the denominator row for columns [a,b)."""
            pv = self.pv
            w = b - a
            dcp = rc_pool.tile([1, 512], f32, name="dcp", tag="dcp", bufs=4)
            nc.vector.tensor_copy(dcp[:, 0:w], pv[64:65, a:b])
            rcp = rc_pool.tile([1, 512], f32, name="rcp", tag="rcp", bufs=4)
            nc.vector.reciprocal_approx_fast(rcp[:, 0:w], dcp[:, 0:w])
            rbc = rc_pool.tile([64, 512], f32, name="rbc", tag="rbc", bufs=4)
            nc.gpsimd.partition_broadcast(rbc[:, 0:w], rcp[:, 0:w], channels=64)
            nc.vector.tensor_mul(
                ot_t[self.hp][self.r0:self.r0 + 64, self.q0 + a:self.q0 + b],
                pv[0:64, a:b], rbc[:, 0:w],
            )

    def attn_pair(ha, hb, half, fillers=()):
        """Two heads, software-pipelined: scores(t) and pv(t-1) per iteration
        so the exp->mask latency is hidden. fillers[t] is a list of thunks
        emitting independent PE work at the end of iteration t."""
        ua, ub = AttnUnit(ha, half), AttnUnit(hb, half)
        n = ua.ki_n
        for t in range(n + 1):
            if t < n:
                ua.emit_scores(t)
                ub.emit_scores(t)
                ua.emit_exp(t)
                ub.emit_exp(t)
            if t >= 1:
                ua.emit_pv(t - 1)
                ub.emit_pv(t - 1)
                if t - 1 == ua.fin0_ki:
                    ua.finish_span(0, 512)
                    ub.finish_span(0, 512)
                if t - 1 == n - 1:
                    ua.finish_span(512, 1024)
                    ub.finish_span(512, 1024)
            if t < len(fillers):
                for fn in fillers[t]:
                    fn()

    oo_box = {}

    def proj_group(s):
        pj = scp.tile([128, 1024], f32, name="sc", tag="sc", bufs=2)
        for nh in range(2):
            for k2 in range(2):
                nc.tensor.matmul(
                    pj[:, nh * 512:(nh + 1) * 512],
                    ot_t[k2][:, s * 128:(s + 1) * 128],
                    wp_t[k2][:, nh * 512:(nh + 1) * 512],
                    start=(k2 == 0), stop=(k2 == 1),
                )
        if s % 2 == 0:
            oo_box[0] = oo_pool.tile([128, 2 * D], bf16, name="oo", tag="oo",
                                     bufs=2)
        oo = oo_box[0]
        nc.vector.tensor_copy(oo[:, (s % 2) * D:(s % 2 + 1) * D], pj[:])
        if s % 2 == 1:
            eng = nc.sync if (s // 2) % 2 == 0 else nc.gpsimd
            eng.dma_start(
                out[(s - 1) * 128:(s + 1) * 128, :].rearrange(
                    "(g p) n -> p g n", p=128),
                oo.rearrange("p (g n) -> p g n", g=2))

    from functools import partial

    # Prelude: exactly what pair (0,1,0) needs to start.
    qkt_group(qt_t, wq_t, 0, 0, 0)
    qkt_group(kt_t, wk_t, 2, 0, 0)
    v_group(0)
    v_group(1)

    attn_pair(0, 1, 0, fillers=[
        [partial(v_group, 2)],
        [partial(v_group, 3)],
        [partial(v_group, 4)],
        [partial(v_group, 5)],
        [partial(v_group, 6)],
        [partial(v_group, 7)],
        [partial(qkt_group, qt_t, wq_t, 0, 1, 0)],
        [partial(qkt_group, kt_t, wk_t, 2, 1, 0)],
        [],
    ])
    attn_pair(2, 3, 0, fillers=[
        [partial(qkt_group, qt_t, wq_t, 0, 0, 1)],
        [partial(qkt_group, kt_t, wk_t, 2, 0, 1)],
        [partial(v_group, 8)],
        [partial(v_group, 9)],
        [partial(v_group, 10)],
        [partial(proj_group, 0)],
        [partial(proj_group, 1)],
        [partial(proj_group, 2)],
        [partial(proj_group, 3)],
    ])
    attn_pair(0, 1, 1, fillers=[
        [partial(qkt_group, qt_t, wq_t, 0, 1, 1)],
        [partial(qkt_group, kt_t, wk_t, 2, 1, 1)],
        [partial(v_group, 11)],
        [partial(v_group, 12)],
        [partial(v_group, 13)],
        [partial(v_group, 14)],
        [partial(v_group, 15)],
        [partial(proj_group, 4)],
        [partial(proj_group, 5)],
    ])
    attn_pair(2, 3, 1, fillers=[
        [partial(proj_group, 6)],
        [partial(proj_group, 7)],
        [], [], [], [], [], [], [], [], [], [], [],
        [partial(proj_group, 8)],
        [partial(proj_group, 9)],
        [partial(proj_group, 10)],
        [partial(proj_group, 11)],
    ])
    for s in range(12, NS):
        proj_group(s)


def _in_maps(x, W_qkv, b_qkv, W_proj):
    bf = ml_dtypes.bfloat16
    maps = []
    # multiplicative causal mask for the transposed diag block: keep k<=q
    mask = np.triu(np.ones((128, 128), np.float32)).astype(bf)
    for core in range(NCORES):
        b, hg = core // 4, core % 4
        cs = slice(hg * FPC, (hg + 1) * FPC)
        bq = b_qkv[cs].astype(np.float32)
        bk = b_qkv[D + hg * FPC: D + (hg + 1) * FPC].astype(np.float32)
        maps.append({
            "xT": np.ascontiguousarray(x[b].T).astype(bf),
            "wq": np.ascontiguousarray(W_qkv[:, cs]).astype(bf),
            "wk": np.ascontiguousarray(W_qkv[:, D + hg * FPC: D + (hg + 1) * FPC]).astype(bf),
            "wv": np.ascontiguousarray(W_qkv[:, 2 * D + hg * FPC: 2 * D + (hg + 1) * FPC]).astype(bf),
            "wp": np.ascontiguousarray(W_proj[hg * FPC:(hg + 1) * FPC, :]).astype(bf),
            "bqk": np.ascontiguousarray(
                np.stack([bq[0:128], bq[128:256], bk[0:128], bk[128:256]], axis=1)),
            "maskT": mask,
        })
    return maps


def get_nc():
    if "nc" not in _CACHE:
        _CACHE["nc"] = _build()
    return _CACHE["nc"]


def _postprocess(partials, b_qkv, W_proj, b_proj):
    out = np.zeros((B, S, D), np.float32)
    for core in range(NCORES):
        out[core // 4] += np.asarray(partials[core], np.float32)
    bv = np.asarray(b_qkv, np.float32)[2 * D:3 * D]
    out += bv @ np.asarray(W_proj, np.float32) + np.asarray(b_proj, np.float32)
    return out


def kernel(x, W_qkv, b_qkv, W_proj, b_proj, _trace=False):
    from concourse.bass_utils import run_bass_kernel_spmd

    x = np.asarray(x, np.float32)
    W_qkv = np.asarray(W_qkv, np.float32)
    b_qkv = np.asarray(b_qkv, np.float32)
    W_proj = np.asarray(W_proj, np.float32)
    b_proj = np.asarray(b_proj, np.float32)

    nc = get_nc()
    maps = _in_maps(x, W_qkv, b_qkv, W_proj)
    res = run_bass_kernel_spmd(nc, maps, list(range(NCORES)), trace=_trace)
    _CACHE["last_result"] = res
    partials = [res.results[c]["out"] for c in range(NCORES)]
    return _postprocess(partials, b_qkv, W_proj, b_proj)


# revision 9
# speedup vs baseline: 1.0416x; 1.0292x over previous
"""Causal multi-head attention (B=2, S=2048, D=1024, H=16, hd=64) on 8 trn2 cores.

Sharding: core c handles batch b = c//4 and head group hg = c%4 (4 heads each).
Each core computes its Q/K/V shard (tensor-parallel columns of W_qkv), causal
attention for its 4 heads with scores held transposed ([s_k, s_q] so the PV
matmul needs no on-chip transposes), and a partial output projection over its
256 rows of W_proj. The host sums the 4 partials per batch and adds the exact
bias terms (softmax rows sum to 1, so attn@(V + 1 bv^T) = attn@V + bv^T; the
b_qkv V-slice and b_proj are applied on the host).

Schedule: input DMAs are split across four engine issue queues so the first
projection matmul starts ~4us in. Attention is software-pipelined: the PV
matmuls for step ki are emitted one iteration after the scores for ki, so the
exp (ACT) -> mask (Pool) latency never stalls the PE. The softmax divide is
done per 512-column PSUM bank as soon as that bank's accumulation stops,
which lets the output projection of the last q-ranges weave into the final
attention chain instead of trailing it. PSUM drains stay on DVE (gpsimd has
no PSUM port); SBUF-only work (causal mask, denominator broadcast, V spread)
runs on the otherwise-idle gpsimd.
"""

import numpy as np
import ml_dtypes
from contextlib import ExitStack

B, S, D, H = 2, 2048, 1024, 16
HD = 64
NCORES = 8
FPC = 256  # features per core (4 heads x 64)

_CACHE = {}


def _build():
    import concourse.bacc as bacc
    import concourse.tile as tile
    import concourse.mybir as mybir

    f32 = mybir.dt.float32
    bf16 = mybir.dt.bfloat16

    nc = bacc.Bacc("TRN2", target_bir_lowering=False, debug=False, num_devices=NCORES)

    xT = nc.dram_tensor("xT", [D, S], bf16, kind="ExternalInput").ap()
    wq = nc.dram_tensor("wq", [D, FPC], bf16, kind="ExternalInput").ap()
    wk = nc.dram_tensor("wk", [D, FPC], bf16, kind="ExternalInput").ap()
    wv = nc.dram_tensor("wv", [D, FPC], bf16, kind="ExternalInput").ap()
    wp = nc.dram_tensor("wp", [FPC, D], bf16, kind="ExternalInput").ap()
    bqk = nc.dram_tensor("bqk", [128, 4], f32, kind="ExternalInput").ap()
    maskT = nc.dram_tensor("maskT", [128, 128], bf16, kind="ExternalInput").ap()
    out = nc.dram_tensor("out", [S, D], bf16, kind="ExternalOutput").ap()

    with tile.TileContext(nc) as tc:
        with ExitStack() as ctx:
            _body(ctx, tc, mybir, out, xT, wq, wk, wv, wp, bqk, maskT)

    nc.compile()
    return nc


def _body(ctx, tc, mybir, out, xT, wq, wk, wv, wp, bqk, maskT):
    nc = tc.nc
    f32 = mybir.dt.float32
    bf16 = mybir.dt.bfloat16
    AF = mybir.ActivationFunctionType
    NK = D // 128   # 8 contraction tiles for qkv/proj-input dim
    NS = S // 128   # 16 sequence tiles

    sb = ctx.enter_context(tc.tile_pool(name="sb", bufs=1))

    xt_a = sb.tile([128, NK * S], bf16, name="xta", tag="xta")
    xt_t = [xt_a[:, k * S:(k + 1) * S] for k in range(NK)]
    wq_a = sb.tile([128, NK * FPC], bf16, name="wqa", tag="wqa")
    wq_t = [wq_a[:, k * FPC:(k + 1) * FPC] for k in range(NK)]
    wk_a = sb.tile([128, NK * FPC], bf16, name="wka", tag="wka")
    wk_t = [wk_a[:, k * FPC:(k + 1) * FPC] for k in range(NK)]
    wv_a = sb.tile([128, NK * FPC], bf16, name="wva", tag="wva")
    wv_t = [wv_a[:, k * FPC:(k + 1) * FPC] for k in range(NK)]
    wp_a = sb.tile([128, 2 * D], bf16, name="wpa", tag="wpa")
    wp_t = [wp_a[:, k * D:(k + 1) * D] for k in range(2)]
    qt_t = [sb.tile([128, S], bf16, name=f"qtt{f}", tag=f"qtt{f}") for f in range(2)]
    kt_t = [sb.tile([128, S], bf16, name=f"ktt{f}", tag=f"ktt{f}") for f in range(2)]
    v_t = [sb.tile([128, 4 * 65], bf16, name=f"vt{s}", tag=f"vt{s}") for s in range(NS)]
    ot_t = [sb.tile([128, S], bf16, name=f"ott{f}", tag=f"ott{f}") for f in range(2)]
    bqk_t = sb.tile([128, 4], f32, name="bqkt", tag="bqkt")
    mask_t = sb.tile([128, 128], bf16, name="maskt", tag="maskt")

    p_pool = ctx.enter_context(tc.tile_pool(name="pp", bufs=1))
    vs_pool = ctx.enter_context(tc.tile_pool(name="vsp", bufs=2))
    rc_pool = ctx.enter_context(tc.tile_pool(name="rcp", bufs=4))
    oo_pool = ctx.enter_context(tc.tile_pool(name="oop", bufs=2))

    # ---- input DMAs split across four issue queues; each queue's order puts
    # the first attention pair's dependencies first.
    xt3 = xt_a.rearrange("p (k s) -> p k s", k=NK)
    xs3 = xT.rearrange("(k p) s -> p k s", p=128)
    # DMA issue is only possible from SP (sync), Activation (scalar), gpsimd.
    # The critical path for the first matmuls (wq, then the x first-half
    # k-chunks in arrival order) rides the fastest-starting queue (sync).
    # gpsimd's DGE pays a ~6us ucode load, so it only gets late-needed data.
    nc.sync.dma_start(wq_a.rearrange("p (k f) -> p k f", k=NK),
                      wq.rearrange("(k p) f -> p k f", p=128))
    nc.sync.dma_start(bqk_t[:], bqk[:])
    for c in range(4):
        nc.sync.dma_start(xt3[:, 2 * c:2 * c + 2, 0:1024],
                          xs3[:, 2 * c:2 * c + 2, 0:1024])
    # scalar: wk (2nd qkt group), mask (first exp), wv (prelude v groups)
    nc.scalar.dma_start(wk_a.rearrange("p (k f) -> p k f", k=NK),
                        wk.rearrange("(k p) f -> p k f", p=128))
    nc.scalar.dma_start(mask_t[:], maskT[:])
    nc.scalar.dma_start(wv_a.rearrange("p (k f) -> p k f", k=NK),
                        wv.rearrange("(k p) f -> p k f", p=128))
    # gpsimd: x second-half (needed from ~pair 2 on), wp (needed by proj)
    nc.gpsimd.dma_start(xt3[:, 0:4, 1024:2048], xs3[:, 0:4, 1024:2048])
    nc.gpsimd.dma_start(xt3[:, 4:8, 1024:2048], xs3[:, 4:8, 1024:2048])
    nc.gpsimd.dma_start(wp_a.rearrange("p (k f) -> p k f", k=2),
                        wp.rearrange("(k p) f -> p k f", p=128))

    # PSUM: "sc" ring (scores + all filler groups) 2 x [128,1024] = 4 banks,
    # "pv" accumulators 2 x [128,1024] = 4 banks.
    scp = ctx.enter_context(tc.tile_pool(name="ps_sc", bufs=2, space="PSUM"))
    pvp = ctx.enter_context(tc.tile_pool(name="ps_pv", bufs=2, space="PSUM"))

    def qkt_group(dst, w_t, bcol, f, c2):
        """One [128,1024] accumulation group of the Q^T/K^T projection."""
        ps = scp.tile([128, 1024], f32, name="sc", tag="sc", bufs=2)
        for k in range(NK):
            for sp in range(2):
                nc.tensor.matmul(
                    ps[:, sp * 512:(sp + 1) * 512],
                    w_t[k][:, f * 128:(f + 1) * 128],
                    xt_t[k][:, c2 * 1024 + sp * 512: c2 * 1024 + (sp + 1) * 512],
                    start=(k == 0), stop=(k == NK - 1),
                )
        nc.vector.tensor_scalar_add(
            dst[f][:, c2 * 1024:(c2 + 1) * 1024], ps[:],
            bqk_t[:, bcol + f: bcol + f + 1],
        )

    def v_group(s):
        psv = scp.tile([128, FPC], f32, name="sc", tag="sc", bufs=2)
        for k in range(NK):
            nc.tensor.matmul(
                psv[:],
                xt_t[k][:, s * 128:(s + 1) * 128],
                wv_t[k][:],
                start=(k == 0), stop=(k == NK - 1),
            )
        v3 = v_t[s].rearrange("p (h c) -> p h c", h=4)
        nc.vector.tensor_copy(v3[:, :, 0:64],
                              psv.rearrange("p (h c) -> p h c", h=4)[:])
        nc.vector.memset(v3[:, :, 64:65], 1.0)

    class AttnUnit:
        """Causal attention for head h over queries [half*1024, +1024)."""

        def __init__(self, h, half):
            self.h, self.half = h, half
            self.hp, self.hh = h // 2, h % 2
            self.r0 = self.hh * 64
            self.q0 = half * 1024
            self.ki_n = NS // 2 * (half + 1)
            self.fin0_ki = (self.q0 + 511) // 128  # bank0 stops after this ki
            self.pv = pvp.tile([128, 1024], f32, name="pv", tag="pv", bufs=2)
            self.P = {}
            self.spans = {}

        def emit_scores(self, ki):
            q0, r0 = self.q0, self.r0
            qt, kt = qt_t[self.hp], kt_t[self.hp]
            qs = max(ki * 128, q0)   # first unmasked q for this k block
            a0 = qs - q0             # local col offset in the 1024 tile
            diag = ki * 128 >= q0    # diagonal block lives in this half
            spans = [(a0, 512), (512, 1024)] if a0 < 512 else [(a0, 1024)]
            self.spans[ki] = (a0, diag, spans)
            sc = scp.tile([128, 1024], f32, name="sc", tag="sc", bufs=2)
            for (a, b) in spans:
                nc.tensor.matmul(
                    sc[:, a:b],
                    kt[r0:r0 + 64, ki * 128:(ki + 1) * 128],
                    qt[r0:r0 + 64, q0 + a:q0 + b],
                    start=True, stop=True,
                )
            self.sc = sc

        def emit_exp(self, ki):
            a0, diag, _ = self.spans[ki]
            P = p_pool.tile([128, 1024], bf16, name="P", tag="P", bufs=6)
            nc.scalar.activation(P[:, a0:1024], self.sc[:, a0:1024], AF.Exp,
                                 scale=float(HD) ** -0.5)
            if diag:  # causal mask on the diagonal block
                nc.vector.tensor_mul(P[:, a0:a0 + 128],
                                     P[:, a0:a0 + 128], mask_t[:])
            self.P[ki] = P

        def emit_pv(self, ki):
            _, _, spans = self.spans[ki]
            P = self.P.pop(ki)
            for (a, b) in spans:
                # last k-block contributing to this psum bank
                last_ki = min(self.ki_n - 1, (self.q0 + b - 1) // 128)
                nc.tensor.matmul(
                    self.pv[0:65, a:b],
                    v_t[ki][:, self.h * 65:self.h * 65 + 65],
                    P[:, a:b],
                    start=(ki == 0), stop=(ki == last_ki),
                )

        def finish_span(self, a, b):
            """Divide pv rows by the denominator row for columns [a,b)."""
            pv = self.pv
            w = b - a
            dcp = rc_pool.tile([1, 512], f32, name="dcp", tag="dcp", bufs=4)
            nc.vector.tensor_copy(dcp[:, 0:w], pv[64:65, a:b])
            rcp = rc_pool.tile([1, 512], f32, name="rcp", tag="rcp", bufs=4)
            nc.vector.reciprocal_approx_fast(rcp[:, 0:w], dcp[:, 0:w])
            rbc = rc_pool.tile([64, 512], f32, name="rbc", tag="rbc", bufs=4)
            nc.gpsimd.partition_broadcast(rbc[:, 0:w], rcp[:, 0:w], channels=64)
            nc.vector.tensor_mul(
                ot_t[self.hp][self.r0:self.r0 + 64, self.q0 + a:self.q0 + b],
                pv[0:64, a:b], rbc[:, 0:w],
            )

    def attn_pair(ha, hb, half, fillers=()):
        """Two heads, software-pipelined: scores(t) and pv(t-1) per iteration
        so the exp->mask latency is hidden. fillers[t] is a list of thunks
        emitting independent PE work at the end of iteration t."""
        ua, ub = AttnUnit(ha, half), AttnUnit(hb, half)
        n = ua.ki_n
        for t in range(n + 1):
            if t < n:
                ua.emit_scores(t)
                ub.emit_scores(t)
                ua.emit_exp(t)
                ub.emit_exp(t)
            if t >= 1:
                ua.emit_pv(t - 1)
                ub.emit_pv(t - 1)
                if t - 1 == ua.fin0_ki:
                    ua.finish_span(0, 512)
                    ub.finish_span(0, 512)
                if t - 1 == n - 1:
                    ua.finish_span(512, 1024)
                    ub.finish_span(512, 1024)
            if t < len(fillers):
                for fn in fillers[t]:
                    fn()

    def proj_group(s):
        pj = scp.tile([128, 1024], f32, name="sc", tag="sc", bufs=2)
        for nh in range(2):
            for k2 in range(2):
                nc.tensor.matmul(
                    pj[:, nh * 512:(nh + 1) * 512],
                    ot_t[k2][:, s * 128:(s + 1) * 128],
                    wp_t[k2][:, nh * 512:(nh + 1) * 512],
                    start=(k2 == 0), stop=(k2 == 1),
                )
        oo = oo_pool.tile([128, D], bf16, name="oo", tag="oo", bufs=3)
        nc.vector.tensor_copy(oo[:], pj[:])
        # one DMA per 128-row group so the drain starts immediately; the two
        # final groups go out on gpsimd in parallel with sync's
        eng = nc.gpsimd if s in (13, 15) else nc.sync
        eng.dma_start(out[s * 128:(s + 1) * 128, :], oo[:])

    from functools import partial

    # Prelude: exactly what pair (0,1,0) needs to start.
    qkt_group(qt_t, wq_t, 0, 0, 0)
    qkt_group(kt_t, wk_t, 2, 0, 0)
    v_group(0)
    v_group(1)

    attn_pair(0, 1, 0, fillers=[
        [partial(v_group, 2)],
        [partial(v_group, 3)],
        [partial(v_group, 4)],
        [partial(v_group, 5)],
        [partial(v_group, 6)],
        [partial(v_group, 7)],
        [partial(qkt_group, qt_t, wq_t, 0, 1, 0)],
        [partial(qkt_group, kt_t, wk_t, 2, 1, 0)],
        [],
    ])
    attn_pair(2, 3, 0, fillers=[
        [partial(qkt_group, qt_t, wq_t, 0, 0, 1)],
        [partial(qkt_group, kt_t, wk_t, 2, 0, 1)],
        [partial(v_group, 8)],
        [],
        [partial(v_group, 9)],
        [],
        [partial(v_group, 10)],
        [partial(proj_group, 0)],
        [partial(proj_group, 1)],
    ])
    attn_pair(0, 1, 1, fillers=[
        [partial(qkt_group, qt_t, wq_t, 0, 1, 1)],
        [],
        [partial(qkt_group, kt_t, wk_t, 2, 1, 1)],
        [],
        [partial(v_group, 11)],
        [],
        [partial(v_group, 12)],
        [],
        [partial(v_group, 13)],
        [partial(v_group, 14)],
        [partial(v_group, 15)],
        [],
        [partial(proj_group, 2)],
        [],
        [partial(proj_group, 3)],
        [],
        [partial(proj_group, 4)],
    ])
    attn_pair(2, 3, 1, fillers=[
        [partial(proj_group, 5)],
        [],
        [partial(proj_group, 6)],
        [],
        [partial(proj_group, 7)],
        [], [], [], [], [], [], [], [],
        [partial(proj_group, 8)],
        [partial(proj_group, 9)],
        [partial(proj_group, 10)],
        [partial(proj_group, 11)],
    ])
    for s in range(12, NS):
        proj_group(s)


def _in_maps(x, W_qkv, b_qkv, W_proj):
    bf = ml_dtypes.bfloat16
    maps = []
    # multiplicative causal mask for the transposed diag block: keep k<=q
    mask = np.triu(np.ones((128, 128), np.float32)).astype(bf)
    for core in range(NCORES):
        b, hg = core // 4, core % 4
        cs = slice(hg * FPC, (hg + 1) * FPC)
        bq = b_qkv[cs].astype(np.float32)
        bk = b_qkv[D + hg * FPC: D + (hg + 1) * FPC].astype(np.float32)
        maps.append({
            "xT": np.ascontiguousarray(x[b].T).astype(bf),
            "wq": np.ascontiguousarray(W_qkv[:, cs]).astype(bf),
            "wk": np.ascontiguousarray(W_qkv[:, D + hg * FPC: D + (hg + 1) * FPC]).astype(bf),
            "wv": np.ascontiguousarray(W_qkv[:, 2 * D + hg * FPC: 2 * D + (hg + 1) * FPC]).astype(bf),
            "wp": np.ascontiguousarray(W_proj[hg * FPC:(hg + 1) * FPC, :]).astype(bf),
            "bqk": np.ascontiguousarray(
                np.stack([bq[0:128], bq[128:256], bk[0:128], bk[128:256]], axis=1)),
            "maskT": mask,
        })
    return maps


def get_nc():
    if "nc" not in _CACHE:
        _CACHE["nc"] = _build()
    return _CACHE["nc"]


def _postprocess(partials, b_qkv, W_proj, b_proj):
    out = np.zeros((B, S, D), np.float32)
    for core in range(NCORES):
        out[core // 4] += np.asarray(partials[core], np.float32)
    bv = np.asarray(b_qkv, np.float32)[2 * D:3 * D]
    out += bv @ np.asarray(W_proj, np.float32) + np.asarray(b_proj, np.float32)
    return out


def kernel(x, W_qkv, b_qkv, W_proj, b_proj, _trace=False):
    from concourse.bass_utils import run_bass_kernel_spmd

    x = np.asarray(x, np.float32)
    W_qkv = np.asarray(W_qkv, np.float32)
    b_qkv = np.asarray(b_qkv, np.float32)
    W_proj = np.asarray(W_proj, np.float32)
    b_proj = np.asarray(b_proj, np.float32)

    nc = get_nc()
    maps = _in_maps(x, W_qkv, b_qkv, W_proj)
    res = run_bass_kernel_spmd(nc, maps, list(range(NCORES)), trace=_trace)
    _CACHE["last_result"] = res
    partials = [res.results[c]["out"] for c in range(NCORES)]
    return _postprocess(partials, b_qkv, W_proj, b_proj)
